# revision 19
# baseline (speedup 1.0000x reference)
"""Trainium2 Bass kernel for nn_CrossAttention (B=8, N=M=1024, D=1024, DK=768, H=16).

Sharding: data-parallel over batch B=8 -> one batch item per NeuronCore.
No collectives; attn.mean(dim=1) is over heads, all heads of a batch item
live on one core.

Per-core layout strategy:
  - Host pre-transposes activations and weights so every matmul has its
    contraction dim on SBUF partitions.
  - Scores are computed transposed: scoresT[m, n]. exp() is applied without
    max-subtraction (scores are bounded ~|2|; softmax value is unchanged).
  - QK^T for a head pair runs row-packed (64-row groups at tile_position
    (0,0)/(64,0)) so the two matmuls share the PE array concurrently.
  - PV uses a ones-augmented stationary [v_h | 1] so psum row 64 yields the
    softmax denominators for free.
  - The head-pair loop is software-pipelined: window p issues QK(p)
    interleaved with PV(p-1) at m-tile granularity to keep the PE stream
    dense; normalization/mean work of pair p-1 drains during window p.
  - exp tiles are combined [128, 2048] (head A cols 0:1024 | head B
    1024:2048) so the attn-mean mul/add chain runs half as many, twice as
    wide DVE ops; mj=7's chain runs on gpsimd to offload the DVE.
  - Softmax denominators: psum Z rows -> [1,2048] sbuf (ACT), DMA-reshaped
    to [128,16] so the reciprocal uses all 128 lanes, DMA'd back to a row,
    then one gpsimd partition_broadcast of the combined [128,2048] rbc.
  - outcat normalization is fused into the PSUM->SBUF eviction via DVE
    scalar_tensor_tensor (pv * rbc -> f16), replacing ACT copy + DVE mul.
  - attn_meanT accumulates per-head-half in acc2 [128,2048] f16; the final
    A+B merge writes f16 directly to DRAM; the host transposes and casts.
  - out = (outcatT * rbc16) @ (16*w_o.T) + b_o  (1/16 folded into the
    normalizer is compensated by scaling w_o.T by 16 on the host).
"""

import sys
import types

sys.path.insert(0, "/opt/trn_rl_repo")
sys.path.insert(0, "/root/.axon_site")

import numpy as np

N_CORES = 8
B, N, M = 8, 1024, 1024
D = 1024      # Q_DIM
DK = 768      # K_DIM
H = 16        # heads
HD = 64       # head dim
SCALE = HD ** -0.5  # 0.125


def _install_ntff_hook():
    """Make trace=True work under axon (antenv.axon_hooks shim)."""
    if "antenv.axon_hooks" in sys.modules:
        return
    try:
        import antenv
        hooks_mod = types.ModuleType("antenv.axon_hooks")
        _hook = [None]
        hooks_mod.set_axon_ntff_profile_hook = lambda h: _hook.__setitem__(0, h)
        hooks_mod.get_axon_ntff_profile_hook = lambda: _hook[0]
        sys.modules["antenv.axon_hooks"] = hooks_mod
        antenv.axon_hooks = hooks_mod
        from trn_agent_boot.trn_boot import _ntff_profile_via_ctypes
        hooks_mod.set_axon_ntff_profile_hook(
            _ntff_profile_via_ctypes("/opt/axon/libaxon_pjrt.so")
        )
    except Exception:
        pass


_CACHE = {}


def build_module():
    if "nc" in _CACHE:
        return _CACHE["nc"]

    import concourse.tile as tile
    import concourse.mybir as mybir
    from concourse import bacc, library_config

    f32 = mybir.dt.float32
    f32r = mybir.dt.float32r
    f16 = mybir.dt.float16
    AF = mybir.ActivationFunctionType
    ALU = mybir.AluOpType

    nc = bacc.Bacc("TRN2", target_bir_lowering=False, debug=False,
                   num_devices=N_CORES)

    # ---- DRAM tensors (per-core shard) ----
    d_qT = nc.dram_tensor("qT_in", [D, N], f16, kind="ExternalInput").ap()
    d_kT = nc.dram_tensor("kT_in", [DK, M], f16, kind="ExternalInput").ap()
    d_vT = nc.dram_tensor("vT_in", [DK, M], f16, kind="ExternalInput").ap()
    d_wqT = nc.dram_tensor("wqT", [D, D], f16, kind="ExternalInput").ap()
    d_wkT = nc.dram_tensor("wkT", [DK, D], f16, kind="ExternalInput").ap()
    d_wvT = nc.dram_tensor("wvT", [DK, D], f16, kind="ExternalInput").ap()
    d_woT = nc.dram_tensor("woT16", [D, D], f16, kind="ExternalInput").ap()
    d_bq = nc.dram_tensor("bq_pp", [128, 8], f32, kind="ExternalInput").ap()
    d_bk = nc.dram_tensor("bk_pp", [128, 8], f32, kind="ExternalInput").ap()
    d_bv = nc.dram_tensor("bv_r", [1, D], f16, kind="ExternalInput").ap()
    d_bo = nc.dram_tensor("bo_r", [1, D], f16, kind="ExternalInput").ap()
    d_ones = nc.dram_tensor("ones_r", [1, 512], f16, kind="ExternalInput").ap()
    d_out = nc.dram_tensor("out", [N, D], f16, kind="ExternalOutput").ap()
    d_amT = nc.dram_tensor("attn_meanT", [M, N], f16, kind="ExternalOutput").ap()

    with tile.TileContext(nc) as tc:
        nc.gpsimd.load_library(library_config.attn)

        # ---------------- persistent pools ----------------
        const = tc.alloc_tile_pool(name="const", bufs=1)
        persist = tc.alloc_tile_pool(name="persist", bufs=1)

        ones_row = const.tile([1, 512], f16, tag="ones", name="ones")
        nc.sync.dma_start(ones_row[:], d_ones[:, :])
        bq_sb = const.tile([128, 8], f32, tag="bq", name="bq")
        bk_sb = const.tile([128, 8], f32, tag="bk", name="bk")

        qT = [persist.tile([128, N], f16, tag=f"qT{j}", name=f"qT{j}")
              for j in range(8)]
        kT = [persist.tile([128, M], f16, tag=f"kT{j}", name=f"kT{j}")
              for j in range(8)]
        v_sb = [persist.tile([128, H, HD + 1], f16, tag=f"v{j}", name=f"v{j}")
                for j in range(8)]
        outcat = [persist.tile([128, N], f16, tag=f"oc{j}", name=f"oc{j}")
                  for j in range(8)]
        acc = [persist.tile([128, N], f16, tag=f"acc{j}", name=f"acc{j}")
               for j in range(8)]

        for j in range(8):
            nc.vector.memset(v_sb[j][:, :, HD:HD + 1], 1.0)

        # ---------------- phase 1: projections ----------------
        wkv = tc.alloc_tile_pool(name="wkv", bufs=1)
        wk_t = [wkv.tile([128, D], f16, tag=f"wk{c}", name=f"wk{c}")
                for c in range(6)]
        wv_t = [wkv.tile([128, D], f16, tag=f"wv{c}", name=f"wv{c}")
                for c in range(6)]
        bv_sb = wkv.tile([1, D], f16, tag="bv", name="bv")

        wqp = tc.alloc_tile_pool(name="wqp", bufs=1)
        wq_t = [wqp.tile([128, D], f16, tag=f"wq{c}", name=f"wq{c}")
                for c in range(8)]

        # --- Q projection: qT[do, n] = wqT-chunks.T @ qT_in ---
        with tc.tile_pool(name="xq", bufs=4) as xp, \
             tc.tile_pool(name="proj_ps", bufs=1, space="PSUM") as pps:
            for nb in range(2):
                pss = [pps.tile([128, 512], f32, tag=f"ps{j}", name=f"ps{j}")
                       for j in range(8)]
                for c in range(8):
                    if nb == 0:
                        nc.sync.dma_start(wq_t[c][:],
                                          d_wqT[c * 128:(c + 1) * 128, :])
                    xt = xp.tile([128, 512], f16, tag="x", name="xt")
                    nc.sync.dma_start(
                        xt[:], d_qT[c * 128:(c + 1) * 128,
                                    nb * 512:(nb + 1) * 512])
                    for j in range(8):
                        nc.tensor.matmul(
                            pss[j][:],
                            wq_t[c][:, j * 128:(j + 1) * 128],
                            xt[:],
                            start=(c == 0), stop=(c == 7))
                if nb == 0:
                    nc.sync.dma_start(bq_sb[:], d_bq[:, :])
                    nc.sync.dma_start(bk_sb[:], d_bk[:, :])
                for j in range(8):
                    nc.scalar.activation(
                        qT[j][:, nb * 512:(nb + 1) * 512], pss[j][:],
                        AF.Identity, bias=bq_sb[:, j:j + 1], scale=1.0)
        wqp.release()


        with tc.tile_pool(name="xkv", bufs=4) as xp, \
             tc.tile_pool(name="proj_ps2", bufs=1, space="PSUM") as pps:
            # --- K projection ---
            for nb in range(2):
                pss = [pps.tile([128, 512], f32, tag=f"ps{j}", name=f"ps{j}")
                       for j in range(8)]
                for c in range(6):
                    if nb == 0:
                        nc.sync.dma_start(wk_t[c][:],
                                          d_wkT[c * 128:(c + 1) * 128, :])
                        nc.sync.dma_start(wv_t[c][:],
                                          d_wvT[c * 128:(c + 1) * 128, :])
                    xt = xp.tile([128, 1024], f16, tag="x", name="xt")
                    nc.sync.dma_start(
                        xt[:, 0:512], d_kT[c * 128:(c + 1) * 128,
                                           nb * 512:(nb + 1) * 512])
                    for j in range(8):
                        nc.tensor.matmul(
                            pss[j][:],
                            wk_t[c][:, j * 128:(j + 1) * 128],
                            xt[:, 0:512],
                            start=(c == 0), stop=(c == 5))
                if nb == 0:
                    nc.sync.dma_start(bv_sb[:], d_bv[:, :])
                for j in range(8):
                    nc.scalar.activation(
                        kT[j][:, nb * 512:(nb + 1) * 512], pss[j][:],
                        AF.Identity, bias=bk_sb[:, j:j + 1], scale=1.0)

            # --- V projection: v[m, do] = vT_in-chunks.T @ wvT (+ b_v) ---
            for ob in range(2):
                pss = [pps.tile([128, 512], f32, tag=f"ps{j}", name=f"ps{j}")
                       for j in range(8)]
                for c in range(6):
                    xt = xp.tile([128, 1024], f16, tag="x", name="xt")
                    nc.sync.dma_start(xt[:], d_vT[c * 128:(c + 1) * 128, :])
                    for mj in range(8):
                        nc.tensor.matmul(
                            pss[mj][:],
                            xt[:, mj * 128:(mj + 1) * 128],
                            wv_t[c][:, ob * 512:(ob + 1) * 512],
                            start=(c == 0), stop=False)
                for mj in range(8):
                    nc.tensor.matmul(
                        pss[mj][:],
                        ones_row[:, 0:128],
                        bv_sb[:, ob * 512:(ob + 1) * 512],
                        start=False, stop=True)
                    nc.scalar.activation(
                        v_sb[mj][:, ob * 8:(ob + 1) * 8, 0:HD],
                        pss[mj][:].rearrange("p (a b) -> p a b", a=8),
                        AF.Copy)
        wkv.release()

        # w_o loaded during attention so O-projection starts without a stall
        wop = tc.alloc_tile_pool(name="wo", bufs=1)
        wo_t = [wop.tile([128, D], f16, tag=f"wo{c}", name=f"wo{c}")
                for c in range(8)]
        bo_sb = wop.tile([1, D], f16, tag="bo", name="bo")
        nc.sync.dma_start(bo_sb[:], d_bo[:, :])
        for c in range(8):
            nc.sync.dma_start(wo_t[c][:], d_woT[c * 128:(c + 1) * 128, :])

        # O-proj staging pool allocated BEFORE attention pools so its SBUF
        # does not overlap exp tiles (would serialize O-proj behind the last
        # pair's mean ops).
        osp = tc.alloc_tile_pool(name="ostage", bufs=2)

        # ---------------- phase 2: attention (software-pipelined pairs) ----
        with tc.tile_pool(name="exp", bufs=2) as expp, \
             tc.tile_pool(name="att_tmp", bufs=1) as tmpp, \
             tc.tile_pool(name="rinvp", bufs=1) as rinvp, \
             tc.tile_pool(name="rbc", bufs=2) as rbcp, \
             tc.tile_pool(name="qk_ps", bufs=1, space="PSUM") as qkps, \
             tc.tile_pool(name="pv_ps", bufs=1, space="PSUM") as pvps:

            exps = {}
            pvts = {}
            deferred = []
            for p in range(9):
                # ---- interleaved PE stream: QK(p) + PV(p-1) ----
                if p < 8:
                    cur = []
                    exps[p] = cur
                for mj in range(8):
                    if p < 8:
                        psAB = qkps.tile([128, 2048], f32, tag="qkAB",
                                         name="qkAB")
                        for nb in range(2):
                            nc.tensor.matmul(
                                psAB[:, nb * 512:(nb + 1) * 512],
                                kT[p][0:64, mj * 128:(mj + 1) * 128],
                                qT[p][0:64, nb * 512:(nb + 1) * 512],
                                start=True, stop=True, tile_position=(0, 0))
                            nc.tensor.matmul(
                                psAB[:, N + nb * 512:N + (nb + 1) * 512],
                                kT[p][64:128, mj * 128:(mj + 1) * 128],
                                qT[p][64:128, nb * 512:(nb + 1) * 512],
                                start=True, stop=True, tile_position=(64, 0))
                    if p >= 1:
                        pvA, pvB = pvts[p - 1]
                        cprev = exps[p - 1]
                        for nb in range(2):
                            nc.tensor.matmul(
                                pvA[:, nb * 512:(nb + 1) * 512],
                                v_sb[mj][:, 2 * (p - 1), :],
                                cprev[mj][:, nb * 512:(nb + 1) * 512],
                                start=(mj == 0), stop=(mj == 7))
                            nc.tensor.matmul(
                                pvB[:, nb * 512:(nb + 1) * 512],
                                v_sb[mj][:, 2 * (p - 1) + 1, :],
                                cprev[mj][:, N + nb * 512:N + (nb + 1) * 512],
                                start=(mj == 0), stop=(mj == 7))
                    if p < 8:
                        # combined exp tile: head A in cols 0:N, B in N:2N
                        et = expp.tile([128, 2 * N], f16, tag=f"exp{mj}",
                                       name=f"exp{mj}", bufs=2)
                        nc.scalar.activation(et[:], psAB[:], AF.Exp,
                                             scale=SCALE)
                        cur.append(et)
                if p < 8:
                    pvts[p] = (
                        pvps.tile([65, 1024], f32, tag="pvA", name="pvA"),
                        pvps.tile([65, 1024], f32, tag="pvB", name="pvB"))

                # ---- drain pair p-1 ----
                if p >= 1:
                    q = p - 1
                    pvA, pvB = pvts[q]
                    cq = exps[q]
                    # softmax denominators -> reciprocals (all-lane layout)
                    sAB = rinvp.tile([1, 2048], f32, tag="sAB", name="sAB")
                    z128 = rinvp.tile([128, 16], f32, tag="z128", name="z128")
                    ri128 = rinvp.tile([128, 16], f32, tag="ri128",
                                       name="ri128")
                    r16c = rinvp.tile([128, 16], f16, tag="r16c", name="r16c")
                    r16row = rinvp.tile([1, 2048], f16, tag="r16row",
                                        name="r16row")
                    nc.scalar.copy(sAB[:, 0:1024], pvA[64:65, :])
                    nc.scalar.copy(sAB[:, 1024:2048], pvB[64:65, :])
                    nc.sync.dma_start(z128[:], sAB[:])
                    nc.vector.reciprocal_approx_fast(out=ri128[:],
                                                     in_=z128[:])
                    nc.vector.tensor_scalar_mul(r16c[:], ri128[:], 1.0 / H)
                    nc.sync.dma_start(r16row[:], r16c[:])
                    rbc2 = rbcp.tile([128, 2 * N], f16, tag="rbc2",
                                     name="rbc2")
                    nc.gpsimd.partition_broadcast(rbc2[:], r16row[:])

                    def mean_mj(mj, q=q, cq=cq, rbc2=rbc2):
                        eng = nc.gpsimd if mj == 7 else nc.vector
                        at = tmpp.tile([128, 2 * N], f16, tag=f"at{mj & 1}",
                                       name="at")
                        eng.tensor_mul(at[:], cq[mj][:], rbc2[:])
                        if q == 0:
                            eng.tensor_add(acc[mj][:], at[:, 0:N],
                                           at[:, N:2 * N])
                        else:
                            eng.tensor_add(acc[mj][:], acc[mj][:], at[:, 0:N])
                            eng.tensor_add(acc[mj][:], acc[mj][:],
                                           at[:, N:2 * N])

                    def norms(q=q, pvA=pvA, pvB=pvB, rbc2=rbc2):
                        # fused PSUM eviction + normalize: outcat = pv * rbc
                        nc.vector.scalar_tensor_tensor(
                            outcat[q][0:64, :], pvA[0:64, :], 1.0,
                            rbc2[0:64, 0:N], ALU.mult, ALU.mult)
                        nc.vector.scalar_tensor_tensor(
                            outcat[q][64:128, :], pvB[0:64, :], 1.0,
                            rbc2[64:128, N:2 * N], ALU.mult, ALU.mult)

                    if p == 7:
                        # defer most pair-6 means past pair-7's norms so the
                        # tail unblocks O-proj as early as possible
                        mean_mj(0)
                        norms()
                        deferred.append(mean_mj)
                        del pvts[q]
                    elif p < 8:
                        mean_mj(0)
                        mean_mj(1)
                        norms()
                        for mj in range(2, 8):
                            mean_mj(mj)
                        del exps[q]
                        del pvts[q]
                    else:
                        # tail: pair-7 norms first so O-proj starts early;
                        # deferred pair-6 means + pair-7 means overlap O-proj
                        norms()
                        dm6 = deferred.pop()
                        for mj in range(1, 8):
                            dm6(mj)
                        del exps[6]
                        for mj in range(8):
                            mean_mj(mj)
                        del exps[q]
                        del pvts[q]

        # ---------------- phase 3: O-projection + outputs ----------------
        with tc.tile_pool(name="o_ps", bufs=4, space="PSUM") as ops:
            for nj in range(8):
                ost = osp.tile([128, D], f16, tag="ost", name="ost")
                for ob in range(2):
                    ps = ops.tile([128, 512], f32, tag="ps", name="ps")
                    for c in range(8):
                        nc.tensor.matmul(
                            ps[:],
                            outcat[c][:, nj * 128:(nj + 1) * 128],
                            wo_t[c][:, ob * 512:(ob + 1) * 512],
                            start=(c == 0), stop=False)
                    nc.tensor.matmul(
                        ps[:],
                        ones_row[:, 0:128],
                        bo_sb[:, ob * 512:(ob + 1) * 512],
                        start=False, stop=True)
                    nc.scalar.copy(ost[:, ob * 512:(ob + 1) * 512], ps[:])
                nc.sync.dma_start(d_out[nj * 128:(nj + 1) * 128, :], ost[:])

            # attn_meanT: f16 accumulator DMA'd straight out
            for mj in range(8):
                nc.sync.dma_start(d_amT[mj * 128:(mj + 1) * 128, :],
                                  acc[mj][:])

        osp.release()
        wop.release()
        persist.release()
        const.release()

    nc.compile()
    _CACHE["nc"] = nc
    return nc


def prepare_in_maps(query, key, value, w_q, b_q, w_k, b_k, w_v, b_v, w_o, b_o):
    """Host-side sharding + layout prep. Returns list of per-core input dicts."""
    f = np.float32
    h16 = np.float16
    wqT = np.ascontiguousarray(np.asarray(w_q, f).T.astype(h16))
    wkT = np.ascontiguousarray(np.asarray(w_k, f).T.astype(h16))
    wvT = np.ascontiguousarray(np.asarray(w_v, f).T.astype(h16))
    woT16 = np.ascontiguousarray(
        (np.asarray(w_o, f).T * np.float32(H)).astype(h16))
    bq_pp = np.ascontiguousarray(np.asarray(b_q, f).reshape(8, 128).T)
    bk_pp = np.ascontiguousarray(np.asarray(b_k, f).reshape(8, 128).T)
    bv_r = np.asarray(b_v, f).reshape(1, D).astype(h16)
    bo_r = np.asarray(b_o, f).reshape(1, D).astype(h16)
    ones_r = np.ones((1, 512), h16)
    query = np.asarray(query, f)
    key = np.asarray(key, f)
    value = np.asarray(value, f)

    in_maps = []
    for b in range(B):
        in_maps.append({
            "qT_in": np.ascontiguousarray(query[b].T.astype(h16)),
            "kT_in": np.ascontiguousarray(key[b].T.astype(h16)),
            "vT_in": np.ascontiguousarray(value[b].T.astype(h16)),
            "wqT": wqT, "wkT": wkT, "wvT": wvT, "woT16": woT16,
            "bq_pp": bq_pp, "bk_pp": bk_pp, "bv_r": bv_r, "bo_r": bo_r,
            "ones_r": ones_r,
        })
    return in_maps


def run(in_maps, trace=False, **kw):
    _install_ntff_hook()
    from concourse.bass_utils import run_bass_kernel_spmd
    nc = build_module()
    return run_bass_kernel_spmd(nc, in_maps, core_ids=list(range(N_CORES)),
                                trace=trace, **kw)


def kernel(query, key, value, w_q, b_q, w_k, b_k, w_v, b_v, w_o, b_o):
    in_maps = prepare_in_maps(query, key, value, w_q, b_q, w_k, b_k,
                              w_v, b_v, w_o, b_o)
    res = run(in_maps)
    out = np.stack(
        [res.results[b]["out"].astype(np.float32) for b in range(B)])
    attn_mean = np.stack(
        [res.results[b]["attn_meanT"].T.astype(np.float32) for b in range(B)])
    return out, attn_mean


# revision 27
# speedup vs baseline: 1.1057x; 1.1057x over previous
"""Trainium2 Bass kernel for nn_CrossAttention (B=8, N=M=1024, D=1024, DK=768, H=16).

Sharding: data-parallel over batch B=8 -> one batch item per NeuronCore.
No collectives; attn.mean(dim=1) is over heads, all heads of a batch item
live on one core.

Per-core layout strategy:
  - Host pre-transposes activations and weights so every matmul has its
    contraction dim on SBUF partitions.
  - Scores are computed transposed: scoresT[m, n]. exp() is applied without
    max-subtraction (scores are bounded ~|2|; softmax value is unchanged).
  - QK^T for a head pair runs row-packed (64-row groups at tile_position
    (0,0)/(64,0)) so the two matmuls share the PE array concurrently.
  - PV uses a ones-augmented stationary [v_h | 1] so psum row 64 yields the
    softmax denominators for free.
  - The head-pair loop is software-pipelined: window p issues QK(p)
    interleaved with PV(p-1) at m-tile granularity to keep the PE stream
    dense; normalization/mean work of pair p-1 drains during window p.
  - exp tiles are combined [128, 2048] (head A cols 0:1024 | head B
    1024:2048) so the attn-mean mul/add chain runs half as many, twice as
    wide DVE ops; mj=7's chain runs on gpsimd to offload the DVE.
  - Softmax denominators: psum Z rows -> [1,2048] sbuf (ACT), DMA-reshaped
    to [128,16] so the reciprocal uses all 128 lanes, DMA'd back to a row,
    then one gpsimd partition_broadcast of the combined [128,2048] rbc.
  - outcat normalization is fused into the PSUM->SBUF eviction via DVE
    scalar_tensor_tensor (pv * rbc -> f16), replacing ACT copy + DVE mul.
  - attn_meanT accumulates per-head-half in acc2 [128,2048] f16; the final
    A+B merge writes f16 directly to DRAM; the host transposes and casts.
  - out = (outcatT * rbc16) @ (16*w_o.T) + b_o  (1/16 folded into the
    normalizer is compensated by scaling w_o.T by 16 on the host).
"""

import sys
import types

sys.path.insert(0, "/opt/trn_rl_repo")
sys.path.insert(0, "/root/.axon_site")

import numpy as np

N_CORES = 8
B, N, M = 8, 1024, 1024
D = 1024      # Q_DIM
DK = 768      # K_DIM
H = 16        # heads
HD = 64       # head dim
SCALE = HD ** -0.5  # 0.125


def _install_ntff_hook():
    """Make trace=True work under axon (antenv.axon_hooks shim)."""
    if "antenv.axon_hooks" in sys.modules:
        return
    try:
        import antenv
        hooks_mod = types.ModuleType("antenv.axon_hooks")
        _hook = [None]
        hooks_mod.set_axon_ntff_profile_hook = lambda h: _hook.__setitem__(0, h)
        hooks_mod.get_axon_ntff_profile_hook = lambda: _hook[0]
        sys.modules["antenv.axon_hooks"] = hooks_mod
        antenv.axon_hooks = hooks_mod
        from trn_agent_boot.trn_boot import _ntff_profile_via_ctypes
        hooks_mod.set_axon_ntff_profile_hook(
            _ntff_profile_via_ctypes("/opt/axon/libaxon_pjrt.so")
        )
    except Exception:
        pass


_CACHE = {}


def build_module():
    if "nc" in _CACHE:
        return _CACHE["nc"]

    import concourse.tile as tile
    import concourse.mybir as mybir
    from concourse import bacc, library_config

    f32 = mybir.dt.float32
    f32r = mybir.dt.float32r
    f16 = mybir.dt.float16
    AF = mybir.ActivationFunctionType
    ALU = mybir.AluOpType

    nc = bacc.Bacc("TRN2", target_bir_lowering=False, debug=False,
                   num_devices=N_CORES)

    # ---- DRAM tensors (per-core shard) ----
    d_qT = nc.dram_tensor("qT_in", [D, N], f16, kind="ExternalInput").ap()
    d_kT = nc.dram_tensor("kT_in", [DK, M], f16, kind="ExternalInput").ap()
    d_vT = nc.dram_tensor("vT_in", [DK, M], f16, kind="ExternalInput").ap()
    d_wqT = nc.dram_tensor("wqT", [D, D], f16, kind="ExternalInput").ap()
    d_wkT = nc.dram_tensor("wkT", [DK, D], f16, kind="ExternalInput").ap()
    d_wvT = nc.dram_tensor("wvT", [DK, D], f16, kind="ExternalInput").ap()
    d_woT = nc.dram_tensor("woT16", [D, D], f16, kind="ExternalInput").ap()
    d_bq = nc.dram_tensor("bq_pp", [128, 8], f32, kind="ExternalInput").ap()
    d_bk = nc.dram_tensor("bk_pp", [128, 8], f32, kind="ExternalInput").ap()
    d_bv = nc.dram_tensor("bv_r", [1, D], f16, kind="ExternalInput").ap()
    d_bo = nc.dram_tensor("bo_r", [1, D], f16, kind="ExternalInput").ap()
    d_ones = nc.dram_tensor("ones_r", [1, 512], f16, kind="ExternalInput").ap()
    d_out = nc.dram_tensor("out", [N, D], f16, kind="ExternalOutput").ap()
    d_amT = nc.dram_tensor("attn_meanT", [M, N], f16, kind="ExternalOutput").ap()

    with tile.TileContext(nc) as tc:
        nc.gpsimd.load_library(library_config.attn)

        # ---------------- persistent pools ----------------
        const = tc.alloc_tile_pool(name="const", bufs=1)
        persist = tc.alloc_tile_pool(name="persist", bufs=1)

        ones_row = const.tile([1, 128], f16, tag="ones", name="ones")
        nc.sync.dma_start(ones_row[:], d_ones[:, 0:128])
        bq_sb = const.tile([128, 8], f32, tag="bq", name="bq")
        bk_sb = const.tile([128, 8], f32, tag="bk", name="bk")

        qT = [persist.tile([128, N], f16, tag=f"qT{j}", name=f"qT{j}")
              for j in range(8)]
        kT = [persist.tile([128, M], f16, tag=f"kT{j}", name=f"kT{j}")
              for j in range(8)]
        v_sb = [persist.tile([128, H, HD + 1], f16, tag=f"v{j}", name=f"v{j}")
                for j in range(8)]
        outcat = [persist.tile([128, N], f16, tag=f"oc{j}", name=f"oc{j}")
                  for j in range(8)]
        acc = [persist.tile([128, N], f16, tag=f"acc{j}", name=f"acc{j}")
               for j in range(8)]

        # ones column scaled by H so the PV row-64 sums come out as H*Z and
        # their reciprocal is directly r = 1/(H*Z) (w_o is pre-scaled by H).
        for j in range(8):
            nc.vector.memset(v_sb[j][:, :, HD:HD + 1], float(H))

        # ---------------- phase 1: projections ----------------
        wkv = tc.alloc_tile_pool(name="wkv", bufs=1)
        wk_t = [wkv.tile([128, D], f16, tag=f"wk{c}", name=f"wk{c}")
                for c in range(6)]
        wv_t = [wkv.tile([128, D], f16, tag=f"wv{c}", name=f"wv{c}")
                for c in range(6)]
        bv_sb = wkv.tile([1, D], f16, tag="bv", name="bv")

        wqp = tc.alloc_tile_pool(name="wqp", bufs=1)
        wq_t = [wqp.tile([128, D], f16, tag=f"wq{c}", name=f"wq{c}")
                for c in range(8)]

        # --- Q projection: qT[do, n] = wqT-chunks.T @ qT_in ---
        with tc.tile_pool(name="xq", bufs=4) as xp, \
             tc.tile_pool(name="proj_ps", bufs=1, space="PSUM") as pps:
            for nb in range(2):
                pss = [pps.tile([128, 512], f32, tag=f"ps{j}", name=f"ps{j}")
                       for j in range(8)]
                for c in range(8):
                    if nb == 0:
                        nc.sync.dma_start(wq_t[c][:],
                                          d_wqT[c * 128:(c + 1) * 128, :])
                    xt = xp.tile([128, 512], f16, tag="x", name="xt")
                    nc.sync.dma_start(
                        xt[:], d_qT[c * 128:(c + 1) * 128,
                                    nb * 512:(nb + 1) * 512])
                    for j in range(8):
                        nc.tensor.matmul(
                            pss[j][:],
                            wq_t[c][:, j * 128:(j + 1) * 128],
                            xt[:],
                            start=(c == 0), stop=(c == 7))
                if nb == 0:
                    nc.sync.dma_start(bq_sb[:], d_bq[:, :])
                    nc.sync.dma_start(bk_sb[:], d_bk[:, :])
                for j in range(8):
                    nc.scalar.activation(
                        qT[j][:, nb * 512:(nb + 1) * 512], pss[j][:],
                        AF.Identity, bias=bq_sb[:, j:j + 1], scale=1.0)
        wqp.release()


        with tc.tile_pool(name="xkv", bufs=4) as xp, \
             tc.tile_pool(name="proj_ps2", bufs=1, space="PSUM") as pps:
            # --- K projection ---
            for nb in range(2):
                pss = [pps.tile([128, 512], f32, tag=f"ps{j}", name=f"ps{j}")
                       for j in range(8)]
                for c in range(6):
                    if nb == 0:
                        nc.sync.dma_start(wk_t[c][:],
                                          d_wkT[c * 128:(c + 1) * 128, :])
                        nc.sync.dma_start(wv_t[c][:],
                                          d_wvT[c * 128:(c + 1) * 128, :])
                    xt = xp.tile([128, 1024], f16, tag="x", name="xt")
                    nc.sync.dma_start(
                        xt[:, 0:512], d_kT[c * 128:(c + 1) * 128,
                                           nb * 512:(nb + 1) * 512])
                    for j in range(8):
                        nc.tensor.matmul(
                            pss[j][:],
                            wk_t[c][:, j * 128:(j + 1) * 128],
                            xt[:, 0:512],
                            start=(c == 0), stop=(c == 5))
                if nb == 0:
                    nc.sync.dma_start(bv_sb[:], d_bv[:, :])
                for j in range(8):
                    nc.scalar.activation(
                        kT[j][:, nb * 512:(nb + 1) * 512], pss[j][:],
                        AF.Identity, bias=bk_sb[:, j:j + 1], scale=1.0)

            # --- V projection: v[m, do] = vT_in-chunks.T @ wvT (+ b_v) ---
            for ob in range(2):
                pss = [pps.tile([128, 512], f32, tag=f"ps{j}", name=f"ps{j}")
                       for j in range(8)]
                for c in range(6):
                    xt = xp.tile([128, 1024], f16, tag="x", name="xt")
                    nc.sync.dma_start(xt[:], d_vT[c * 128:(c + 1) * 128, :])
                    for mj in range(8):
                        nc.tensor.matmul(
                            pss[mj][:],
                            xt[:, mj * 128:(mj + 1) * 128],
                            wv_t[c][:, ob * 512:(ob + 1) * 512],
                            start=(c == 0), stop=False)
                for mj in range(8):
                    nc.tensor.matmul(
                        pss[mj][:],
                        ones_row[:, 0:128],
                        bv_sb[:, ob * 512:(ob + 1) * 512],
                        start=False, stop=True)
                    nc.scalar.activation(
                        v_sb[mj][:, ob * 8:(ob + 1) * 8, 0:HD],
                        pss[mj][:].rearrange("p (a b) -> p a b", a=8),
                        AF.Copy)
        wkv.release()

        # w_o loaded during attention so O-projection starts without a stall
        wop = tc.alloc_tile_pool(name="wo", bufs=1)
        wo_t = [wop.tile([128, D], f16, tag=f"wo{c}", name=f"wo{c}")
                for c in range(8)]
        bo_sb = wop.tile([1, D], f16, tag="bo", name="bo")
        nc.sync.dma_start(bo_sb[:], d_bo[:, :])
        for c in range(8):
            nc.sync.dma_start(wo_t[c][:], d_woT[c * 128:(c + 1) * 128, :])

        # O-proj staging pool allocated BEFORE attention pools so its SBUF
        # does not overlap exp tiles (would serialize O-proj behind the last
        # pair's mean ops).
        osp = tc.alloc_tile_pool(name="ostage", bufs=2)

        # ---------------- phase 2: attention (software-pipelined pairs) ----
        # Pipeline (window p): QK(p)+exp(p); PV(p-1) front-loaded so its Z
        # row is ready mid-window; recip(p-1) mid-window on DVE; bc(p-1) on
        # gpsimd late-window; means(p-2) run from window top (their rbc was
        # finished a window ago).  exp tags for the late-draining mj get a
        # third buffer so next-next window's exp doesn't stall on the mean
        # backlog.
        with tc.tile_pool(name="exp", bufs=2) as expp, \
             tc.tile_pool(name="att_tmp", bufs=1) as tmpp, \
             tc.tile_pool(name="rinvp", bufs=1) as rinvp, \
             tc.tile_pool(name="rbc", bufs=2) as rbcp, \
             tc.tile_pool(name="qk_ps", bufs=2, space="PSUM") as qkps, \
             tc.tile_pool(name="pv_ps", bufs=1, space="PSUM") as pvps:

            exps = {}
            pvts = {}
            rbcs = {}

            def emit_qk_mj(p, mj):
                tA = qkps.tile([128, 1024], f32, tag="qk", name="qkA")
                tB = qkps.tile([128, 1024], f32, tag="qk", name="qkB")
                for nb in range(2):
                    nc.tensor.matmul(
                        tA[:, nb * 512:(nb + 1) * 512],
                        kT[p][0:64, mj * 128:(mj + 1) * 128],
                        qT[p][0:64, nb * 512:(nb + 1) * 512],
                        start=True, stop=True, tile_position=(0, 0))
                    nc.tensor.matmul(
                        tB[:, nb * 512:(nb + 1) * 512],
                        kT[p][64:128, mj * 128:(mj + 1) * 128],
                        qT[p][64:128, nb * 512:(nb + 1) * 512],
                        start=True, stop=True, tile_position=(64, 0))
                et = expp.tile([128, 2 * N], f16, tag=f"exp{mj}",
                               name=f"exp{mj}",
                               bufs=3 if mj == 6 else 2)
                nc.scalar.activation(et[:, 0:N], tA[:], AF.Exp, scale=SCALE)
                nc.scalar.activation(et[:, N:2 * N], tB[:], AF.Exp,
                                     scale=SCALE)
                exps[p].append(et)

            def emit_pv(q):
                pv = pvps.tile([65, 2048], f32, tag="pv", name="pv")
                pvts[q] = pv
                cq = exps[q]
                for mj in range(8):
                    for nb in range(2):
                        nc.tensor.matmul(
                            pv[:, nb * 512:(nb + 1) * 512],
                            v_sb[mj][:, 2 * q, :],
                            cq[mj][:, nb * 512:(nb + 1) * 512],
                            start=(mj == 0), stop=(mj == 7))
                        nc.tensor.matmul(
                            pv[:, N + nb * 512:N + (nb + 1) * 512],
                            v_sb[mj][:, 2 * q + 1, :],
                            cq[mj][:, N + nb * 512:N + (nb + 1) * 512],
                            start=(mj == 0), stop=(mj == 7))

            def emit_mean(q, mj):
                eng = nc.gpsimd if mj == 7 else nc.vector
                at = tmpp.tile([128, 2 * N], f16,
                               tag="at_gp" if mj == 7 else "at", name="at")
                eng.tensor_mul(at[:], exps[q][mj][:], rbcs[q][:])
                if q == 0:
                    eng.tensor_add(acc[mj][:], at[:, 0:N], at[:, N:2 * N])
                else:
                    eng.tensor_add(acc[mj][:], acc[mj][:], at[:, 0:N])
                    eng.tensor_add(acc[mj][:], acc[mj][:], at[:, N:2 * N])

            r32s = {}
            for p in range(10):
                if p < 8:
                    exps[p] = []
                    emit_qk_mj(p, 0)
                    emit_qk_mj(p, 1)
                if 1 <= p <= 8:
                    emit_pv(p - 1)
                if p < 8:
                    for mj in range(2, 8):
                        emit_qk_mj(p, mj)

                # means of pair p-2 (their rbc completed last window);
                # first batch keeps the DVE FIFO busy until PV(p-1) stops.
                if p >= 2:
                    for mj in range(0, 4):
                        emit_mean(p - 2, mj)
                    emit_mean(p - 2, 7)

                if 1 <= p <= 8:
                    q = p - 1
                    sAB = rinvp.tile([1, 2048], f32, tag="sAB", name="sAB")
                    nc.scalar.copy(sAB[:], pvts[q][64:65, :])
                    r32 = rinvp.tile([1, 2048], f32, tag="r32", name="r32")
                    nc.vector.reciprocal_approx_fast(out=r32[:], in_=sAB[:])
                    r32s[q] = r32

                if p >= 2:
                    for mj in range(4, 7):
                        emit_mean(p - 2, mj)
                    del exps[p - 2]

                if 1 <= p <= 8:
                    q = p - 1
                    r16 = rinvp.tile([1, 2048], f16, tag="r16", name="r16")
                    nc.vector.tensor_scalar_mul(r16[:], r32s[q][:], 1.0)
                    rbc2 = rbcp.tile([128, 2 * N], f16, tag="rbc2",
                                     name="rbc2")
                    nc.gpsimd.partition_broadcast(rbc2[:], r16[:])
                    rbcs[q] = rbc2
                    # outcat: evict on ACT, normalize in place on gpsimd
                    pv = pvts[q]
                    nc.scalar.copy(outcat[q][0:64, :], pv[0:64, 0:N])
                    nc.scalar.copy(outcat[q][64:128, :], pv[0:64, N:2 * N])
                    nc.gpsimd.tensor_mul(outcat[q][0:64, :],
                                         outcat[q][0:64, :],
                                         rbc2[0:64, 0:N])
                    nc.gpsimd.tensor_mul(outcat[q][64:128, :],
                                         outcat[q][64:128, :],
                                         rbc2[64:128, N:2 * N])
                    del pvts[q]

        # ---------------- phase 3: O-projection + outputs ----------------
        with tc.tile_pool(name="o_ps", bufs=4, space="PSUM") as ops:
            for nj in range(8):
                for ob in range(2):
                    ps = ops.tile([128, 512], f32, tag="ps", name="ps")
                    for c in range(8):
                        nc.tensor.matmul(
                            ps[:],
                            outcat[c][:, nj * 128:(nj + 1) * 128],
                            wo_t[c][:, ob * 512:(ob + 1) * 512],
                            start=(c == 0), stop=False)
                    nc.tensor.matmul(
                        ps[:],
                        ones_row[:, 0:128],
                        bo_sb[:, ob * 512:(ob + 1) * 512],
                        start=False, stop=True)
                    ost = osp.tile([128, 512], f16, tag="ost", name="ost")
                    nc.scalar.copy(ost[:], ps[:])
                    nc.sync.dma_start(
                        d_out[nj * 128:(nj + 1) * 128,
                              ob * 512:(ob + 1) * 512], ost[:])

            # attn_meanT: f16 accumulator DMA'd straight out
            for mj in range(8):
                nc.sync.dma_start(d_amT[mj * 128:(mj + 1) * 128, :],
                                  acc[mj][:])

        osp.release()
        wop.release()
        persist.release()
        const.release()

    nc.compile()
    _CACHE["nc"] = nc
    return nc


def prepare_in_maps(query, key, value, w_q, b_q, w_k, b_k, w_v, b_v, w_o, b_o):
    """Host-side sharding + layout prep. Returns list of per-core input dicts."""
    f = np.float32
    h16 = np.float16
    wqT = np.ascontiguousarray(np.asarray(w_q, f).T.astype(h16))
    wkT = np.ascontiguousarray(np.asarray(w_k, f).T.astype(h16))
    wvT = np.ascontiguousarray(np.asarray(w_v, f).T.astype(h16))
    woT16 = np.ascontiguousarray(
        (np.asarray(w_o, f).T * np.float32(H)).astype(h16))
    bq_pp = np.ascontiguousarray(np.asarray(b_q, f).reshape(8, 128).T)
    bk_pp = np.ascontiguousarray(np.asarray(b_k, f).reshape(8, 128).T)
    bv_r = np.asarray(b_v, f).reshape(1, D).astype(h16)
    bo_r = np.asarray(b_o, f).reshape(1, D).astype(h16)
    ones_r = np.ones((1, 512), h16)
    query = np.asarray(query, f)
    key = np.asarray(key, f)
    value = np.asarray(value, f)

    in_maps = []
    for b in range(B):
        in_maps.append({
            "qT_in": np.ascontiguousarray(query[b].T.astype(h16)),
            "kT_in": np.ascontiguousarray(key[b].T.astype(h16)),
            "vT_in": np.ascontiguousarray(value[b].T.astype(h16)),
            "wqT": wqT, "wkT": wkT, "wvT": wvT, "woT16": woT16,
            "bq_pp": bq_pp, "bk_pp": bk_pp, "bv_r": bv_r, "bo_r": bo_r,
            "ones_r": ones_r,
        })
    return in_maps


def run(in_maps, trace=False, **kw):
    _install_ntff_hook()
    from concourse.bass_utils import run_bass_kernel_spmd
    nc = build_module()
    return run_bass_kernel_spmd(nc, in_maps, core_ids=list(range(N_CORES)),
                                trace=trace, **kw)


def kernel(query, key, value, w_q, b_q, w_k, b_k, w_v, b_v, w_o, b_o):
    in_maps = prepare_in_maps(query, key, value, w_q, b_q, w_k, b_k,
                              w_v, b_v, w_o, b_o)
    res = run(in_maps)
    out = np.stack(
        [res.results[b]["out"].astype(np.float32) for b in range(B)])
    attn_mean = np.stack(
        [res.results[b]["attn_meanT"].T.astype(np.float32) for b in range(B)])
    return out, attn_mean


# revision 29
# speedup vs baseline: 1.3312x; 1.2039x over previous
"""Trainium2 Bass kernel for nn_CrossAttention (B=8, N=M=1024, D=1024, DK=768, H=16).

Sharding: data-parallel over batch B=8 -> one batch item per NeuronCore.
No collectives; attn.mean(dim=1) is over heads, all heads of a batch item
live on one core.

Per-core layout strategy:
  - Host pre-transposes activations and weights so every matmul has its
    contraction dim on SBUF partitions.
  - Scores are computed transposed: scoresT[m, n]. exp() is applied without
    max-subtraction (scores are bounded ~|2|; softmax value is unchanged).
  - QK^T for a head pair runs row-packed (64-row groups at tile_position
    (0,0)/(64,0)) so the two matmuls share the PE array concurrently.
  - PV uses a ones-augmented stationary [v_h | 1] so psum row 64 yields the
    softmax denominators for free.
  - The head-pair loop is software-pipelined: window p issues QK(p)
    interleaved with PV(p-1) at m-tile granularity to keep the PE stream
    dense; normalization/mean work of pair p-1 drains during window p.
  - exp tiles are combined [128, 2048] (head A cols 0:1024 | head B
    1024:2048) so the attn-mean mul/add chain runs half as many, twice as
    wide DVE ops; mj=7's chain runs on gpsimd to offload the DVE.
  - Softmax denominators: psum Z rows -> [1,2048] sbuf (ACT), DMA-reshaped
    to [128,16] so the reciprocal uses all 128 lanes, DMA'd back to a row,
    then one gpsimd partition_broadcast of the combined [128,2048] rbc.
  - outcat normalization is fused into the PSUM->SBUF eviction via DVE
    scalar_tensor_tensor (pv * rbc -> f16), replacing ACT copy + DVE mul.
  - attn_meanT accumulates per-head-half in acc2 [128,2048] f16; the final
    A+B merge writes f16 directly to DRAM; the host transposes and casts.
  - out = (outcatT * rbc16) @ (16*w_o.T) + b_o  (1/16 folded into the
    normalizer is compensated by scaling w_o.T by 16 on the host).
"""

import sys
import types

sys.path.insert(0, "/opt/trn_rl_repo")
sys.path.insert(0, "/root/.axon_site")

import numpy as np

N_CORES = 8
B, N, M = 8, 1024, 1024
D = 1024      # Q_DIM
DK = 768      # K_DIM
H = 16        # heads
HD = 64       # head dim
SCALE = HD ** -0.5  # 0.125


def _install_ntff_hook():
    """Make trace=True work under axon (antenv.axon_hooks shim)."""
    if "antenv.axon_hooks" in sys.modules:
        return
    try:
        import antenv
        hooks_mod = types.ModuleType("antenv.axon_hooks")
        _hook = [None]
        hooks_mod.set_axon_ntff_profile_hook = lambda h: _hook.__setitem__(0, h)
        hooks_mod.get_axon_ntff_profile_hook = lambda: _hook[0]
        sys.modules["antenv.axon_hooks"] = hooks_mod
        antenv.axon_hooks = hooks_mod
        from trn_agent_boot.trn_boot import _ntff_profile_via_ctypes
        hooks_mod.set_axon_ntff_profile_hook(
            _ntff_profile_via_ctypes("/opt/axon/libaxon_pjrt.so")
        )
    except Exception:
        pass


_CACHE = {}


def build_module():
    if "nc" in _CACHE:
        return _CACHE["nc"]

    import concourse.tile as tile
    import concourse.mybir as mybir
    from concourse import bacc, library_config

    f32 = mybir.dt.float32
    f32r = mybir.dt.float32r
    f16 = mybir.dt.float16
    AF = mybir.ActivationFunctionType
    ALU = mybir.AluOpType

    nc = bacc.Bacc("TRN2", target_bir_lowering=False, debug=False,
                   num_devices=N_CORES)

    # ---- DRAM tensors (per-core shard) ----
    d_qT = nc.dram_tensor("qT_in", [D, N], f16, kind="ExternalInput").ap()
    d_kT = nc.dram_tensor("kT_in", [DK, M], f16, kind="ExternalInput").ap()
    d_vT = nc.dram_tensor("vT_in", [DK, M], f16, kind="ExternalInput").ap()
    d_wqT = nc.dram_tensor("wqT", [D, D], f16, kind="ExternalInput").ap()
    d_wkT = nc.dram_tensor("wkT", [DK, D], f16, kind="ExternalInput").ap()
    d_wvT = nc.dram_tensor("wvT", [DK, D], f16, kind="ExternalInput").ap()
    d_woT = nc.dram_tensor("woT16", [D, D], f16, kind="ExternalInput").ap()
    d_bq = nc.dram_tensor("bq_pp", [128, 8], f32, kind="ExternalInput").ap()
    d_bk = nc.dram_tensor("bk_pp", [128, 8], f32, kind="ExternalInput").ap()
    d_bv = nc.dram_tensor("bv_r", [1, D], f16, kind="ExternalInput").ap()
    d_bo = nc.dram_tensor("bo_r", [1, D], f16, kind="ExternalInput").ap()
    d_ones = nc.dram_tensor("ones_r", [1, 512], f16, kind="ExternalInput").ap()
    d_out = nc.dram_tensor("out", [N, D], f16, kind="ExternalOutput").ap()
    d_amT = nc.dram_tensor("attn_meanT", [M, N], f16, kind="ExternalOutput").ap()

    with tile.TileContext(nc) as tc:
        nc.gpsimd.load_library(library_config.attn)

        # ---------------- persistent pools ----------------
        const = tc.alloc_tile_pool(name="const", bufs=1)
        persist = tc.alloc_tile_pool(name="persist", bufs=1)

        ones_row = const.tile([1, 128], f16, tag="ones", name="ones")
        nc.sync.dma_start(ones_row[:], d_ones[:, 0:128])
        bq_sb = const.tile([128, 8], f32, tag="bq", name="bq")
        bk_sb = const.tile([128, 8], f32, tag="bk", name="bk")

        qT = [persist.tile([128, N], f16, tag=f"qT{j}", name=f"qT{j}")
              for j in range(8)]
        kT = [persist.tile([128, M], f16, tag=f"kT{j}", name=f"kT{j}")
              for j in range(8)]
        v_sb = [persist.tile([128, H, HD + 1], f16, tag=f"v{j}", name=f"v{j}")
                for j in range(8)]
        outcat = [persist.tile([128, N], f16, tag=f"oc{j}", name=f"oc{j}")
                  for j in range(8)]
        acc = [persist.tile([128, N], f16, tag=f"acc{j}", name=f"acc{j}")
               for j in range(8)]

        # ones column scaled by H so the PV row-64 sums come out as H*Z and
        # their reciprocal is directly r = 1/(H*Z) (w_o is pre-scaled by H).
        for j in range(8):
            nc.vector.memset(v_sb[j][:, :, HD:HD + 1], float(H))

        # ---------------- phase 1: projections ----------------
        wkv = tc.alloc_tile_pool(name="wkv", bufs=1)
        wk_t = [wkv.tile([128, D], f16, tag=f"wk{c}", name=f"wk{c}")
                for c in range(6)]
        wv_t = [wkv.tile([128, D], f16, tag=f"wv{c}", name=f"wv{c}")
                for c in range(6)]
        bv_sb = wkv.tile([1, D], f16, tag="bv", name="bv")

        wqp = tc.alloc_tile_pool(name="wqp", bufs=1)
        wq_t = [wqp.tile([128, D], f16, tag=f"wq{c}", name=f"wq{c}")
                for c in range(8)]

        # --- Q projection: qT[do, n] = wqT-chunks.T @ qT_in ---
        with tc.tile_pool(name="xq", bufs=4) as xp, \
             tc.tile_pool(name="proj_ps", bufs=1, space="PSUM") as pps:
            for nb in range(2):
                pss = [pps.tile([128, 512], f32, tag=f"ps{j}", name=f"ps{j}")
                       for j in range(8)]
                for c in range(8):
                    if nb == 0:
                        nc.sync.dma_start(wq_t[c][:],
                                          d_wqT[c * 128:(c + 1) * 128, :])
                    xt = xp.tile([128, 512], f16, tag="x", name="xt")
                    nc.sync.dma_start(
                        xt[:], d_qT[c * 128:(c + 1) * 128,
                                    nb * 512:(nb + 1) * 512])
                    for j in range(8):
                        nc.tensor.matmul(
                            pss[j][:],
                            wq_t[c][:, j * 128:(j + 1) * 128],
                            xt[:],
                            start=(c == 0), stop=(c == 7))
                if nb == 0:
                    nc.sync.dma_start(bq_sb[:], d_bq[:, :])
                    nc.sync.dma_start(bk_sb[:], d_bk[:, :])
                for j in range(8):
                    nc.scalar.activation(
                        qT[j][:, nb * 512:(nb + 1) * 512], pss[j][:],
                        AF.Identity, bias=bq_sb[:, j:j + 1], scale=1.0)
        wqp.release()


        with tc.tile_pool(name="xkv", bufs=4) as xp, \
             tc.tile_pool(name="proj_ps2", bufs=1, space="PSUM") as pps:
            # --- K projection ---
            for nb in range(2):
                pss = [pps.tile([128, 512], f32, tag=f"ps{j}", name=f"ps{j}")
                       for j in range(8)]
                for c in range(6):
                    if nb == 0:
                        nc.sync.dma_start(wk_t[c][:],
                                          d_wkT[c * 128:(c + 1) * 128, :])
                        nc.sync.dma_start(wv_t[c][:],
                                          d_wvT[c * 128:(c + 1) * 128, :])
                    xt = xp.tile([128, 1024], f16, tag="x", name="xt")
                    nc.sync.dma_start(
                        xt[:, 0:512], d_kT[c * 128:(c + 1) * 128,
                                           nb * 512:(nb + 1) * 512])
                    for j in range(8):
                        nc.tensor.matmul(
                            pss[j][:],
                            wk_t[c][:, j * 128:(j + 1) * 128],
                            xt[:, 0:512],
                            start=(c == 0), stop=(c == 5))
                if nb == 0:
                    nc.sync.dma_start(bv_sb[:], d_bv[:, :])
                for j in range(8):
                    nc.scalar.activation(
                        kT[j][:, nb * 512:(nb + 1) * 512], pss[j][:],
                        AF.Identity, bias=bk_sb[:, j:j + 1], scale=1.0)

            # --- V projection: v[m, do] = vT_in-chunks.T @ wvT (+ b_v) ---
            for ob in range(2):
                pss = [pps.tile([128, 512], f32, tag=f"ps{j}", name=f"ps{j}")
                       for j in range(8)]
                for c in range(6):
                    xt = xp.tile([128, 1024], f16, tag="x", name="xt")
                    nc.sync.dma_start(xt[:], d_vT[c * 128:(c + 1) * 128, :])
                    for mj in range(8):
                        nc.tensor.matmul(
                            pss[mj][:],
                            xt[:, mj * 128:(mj + 1) * 128],
                            wv_t[c][:, ob * 512:(ob + 1) * 512],
                            start=(c == 0), stop=False)
                for mj in range(8):
                    nc.tensor.matmul(
                        pss[mj][:],
                        ones_row[:, 0:128],
                        bv_sb[:, ob * 512:(ob + 1) * 512],
                        start=False, stop=True)
                    nc.scalar.activation(
                        v_sb[mj][:, ob * 8:(ob + 1) * 8, 0:HD],
                        pss[mj][:].rearrange("p (a b) -> p a b", a=8),
                        AF.Copy)
        wkv.release()

        # w_o loaded during attention so O-projection starts without a stall
        wop = tc.alloc_tile_pool(name="wo", bufs=1)
        wo_t = [wop.tile([128, D], f16, tag=f"wo{c}", name=f"wo{c}")
                for c in range(8)]
        bo_sb = wop.tile([1, D], f16, tag="bo", name="bo")
        nc.sync.dma_start(bo_sb[:], d_bo[:, :])
        for c in range(8):
            nc.sync.dma_start(wo_t[c][:], d_woT[c * 128:(c + 1) * 128, :])

        # O-proj staging pool allocated BEFORE attention pools so its SBUF
        # does not overlap exp tiles (would serialize O-proj behind the last
        # pair's mean ops).
        osp = tc.alloc_tile_pool(name="ostage", bufs=2)

        # ---------------- phase 2: attention (software-pipelined pairs) ----
        # Pipeline (window p): QK(p)+exp(p); PV(p-1) front-loaded so its Z
        # row is ready mid-window; recip(p-1) mid-window on DVE; bc(p-1) on
        # gpsimd late-window; means(p-2) run from window top (their rbc was
        # finished a window ago).  exp tags for the late-draining mj get a
        # third buffer so next-next window's exp doesn't stall on the mean
        # backlog.
        with tc.tile_pool(name="exp", bufs=2) as expp, \
             tc.tile_pool(name="att_tmp", bufs=1) as tmpp, \
             tc.tile_pool(name="rinvp", bufs=1) as rinvp, \
             tc.tile_pool(name="rbc", bufs=2) as rbcp, \
             tc.tile_pool(name="qk_ps", bufs=2, space="PSUM") as qkps, \
             tc.tile_pool(name="pv_ps", bufs=1, space="PSUM") as pvps:

            exps = {}
            pvts = {}
            rbcs = {}

            def emit_qk_mj(p, mj):
                tA = qkps.tile([128, 1024], f32, tag="qk", name="qkA")
                tB = qkps.tile([128, 1024], f32, tag="qk", name="qkB")
                for nb in range(2):
                    nc.tensor.matmul(
                        tA[:, nb * 512:(nb + 1) * 512],
                        kT[p][0:64, mj * 128:(mj + 1) * 128],
                        qT[p][0:64, nb * 512:(nb + 1) * 512],
                        start=True, stop=True, tile_position=(0, 0))
                    nc.tensor.matmul(
                        tB[:, nb * 512:(nb + 1) * 512],
                        kT[p][64:128, mj * 128:(mj + 1) * 128],
                        qT[p][64:128, nb * 512:(nb + 1) * 512],
                        start=True, stop=True, tile_position=(64, 0))
                et = expp.tile([128, 2 * N], f16, tag=f"exp{mj}",
                               name=f"exp{mj}",
                               bufs=3 if mj in (5, 6) else 2)
                nc.scalar.activation(et[:, 0:N], tA[:], AF.Exp, scale=SCALE)
                nc.scalar.activation(et[:, N:2 * N], tB[:], AF.Exp,
                                     scale=SCALE)
                exps[p].append(et)

            def emit_pv(q):
                pv = pvps.tile([65, 2048], f32, tag="pv", name="pv")
                pvts[q] = pv
                cq = exps[q]
                for mj in range(8):
                    for nb in range(2):
                        nc.tensor.matmul(
                            pv[:, nb * 512:(nb + 1) * 512],
                            v_sb[mj][:, 2 * q, :],
                            cq[mj][:, nb * 512:(nb + 1) * 512],
                            start=(mj == 0), stop=(mj == 7))
                        nc.tensor.matmul(
                            pv[:, N + nb * 512:N + (nb + 1) * 512],
                            v_sb[mj][:, 2 * q + 1, :],
                            cq[mj][:, N + nb * 512:N + (nb + 1) * 512],
                            start=(mj == 0), stop=(mj == 7))

            def emit_mean(q, mj):
                at = tmpp.tile([128, 2 * N], f16, tag="at", name="at")
                nc.vector.tensor_mul(at[:], exps[q][mj][:], rbcs[q][:])
                if q == 0:
                    nc.vector.tensor_add(acc[mj][:], at[:, 0:N],
                                         at[:, N:2 * N])
                else:
                    nc.vector.tensor_add(acc[mj][:], acc[mj][:], at[:, 0:N])
                    nc.vector.tensor_add(acc[mj][:], acc[mj][:],
                                         at[:, N:2 * N])

            def emit_norms(q):
                # in-place normalization of the evicted outcat (cheap f16 2x)
                nc.vector.tensor_mul(outcat[q][0:64, :], outcat[q][0:64, :],
                                     rbcs[q][0:64, 0:N])
                nc.vector.tensor_mul(outcat[q][64:128, :],
                                     outcat[q][64:128, :],
                                     rbcs[q][64:128, N:2 * N])

            for p in range(10):
                if p < 8:
                    exps[p] = []
                    emit_qk_mj(p, 0)
                    emit_qk_mj(p, 1)
                if 1 <= p <= 8:
                    emit_pv(p - 1)
                if p < 8:
                    for mj in range(2, 6):
                        emit_qk_mj(p, mj)

                # drain of pair p-1: Z row -> all-lane reciprocal -> rbc.
                # The ACT row-copy is emitted mid-exp-stream so the DVE
                # reciprocal's input is ready when the DVE FIFO reaches it.
                if 1 <= p <= 8:
                    q = p - 1
                    pv = pvts[q]
                    sAB = rinvp.tile([1, 2048], f32, tag="sAB", name="sAB")
                    z128 = rinvp.tile([128, 16], f32, tag="z128", name="z128")
                    ri128 = rinvp.tile([128, 16], f32, tag="ri128",
                                       name="ri128")
                    r16c = rinvp.tile([128, 16], f16, tag="r16c", name="r16c")
                    r16row = rinvp.tile([1, 2048], f16, tag="r16row",
                                        name="r16row")
                    nc.scalar.copy(sAB[:], pv[64:65, :])
                    nc.sync.dma_start(z128[:], sAB[:])
                    # unnormalized outcat eviction (normalized next window)
                    nc.scalar.copy(outcat[q][0:64, :], pv[0:64, 0:N])
                    nc.scalar.copy(outcat[q][64:128, :], pv[0:64, N:2 * N])
                    del pvts[q]

                if p < 8:
                    for mj in range(6, 8):
                        emit_qk_mj(p, mj)

                # means of pair p-2 (their rbc completed last window)
                if p >= 2:
                    emit_norms(p - 2)
                    for mj in range(0, 6):
                        emit_mean(p - 2, mj)

                if 1 <= p <= 8:
                    q = p - 1
                    nc.vector.reciprocal_approx_fast(out=ri128[:],
                                                     in_=z128[:])
                    nc.vector.tensor_scalar_mul(r16c[:], ri128[:], 1.0)
                    nc.sync.dma_start(r16row[:], r16c[:])
                    rbc2 = rbcp.tile([128, 2 * N], f16, tag="rbc2",
                                     name="rbc2")
                    nc.gpsimd.partition_broadcast(rbc2[:], r16row[:])
                    rbcs[q] = rbc2

                if p >= 2:
                    for mj in range(6, 8):
                        emit_mean(p - 2, mj)
                    del exps[p - 2]

        # ---------------- phase 3: O-projection + outputs ----------------
        with tc.tile_pool(name="o_ps", bufs=4, space="PSUM") as ops:
            for nj in range(8):
                for ob in range(2):
                    ps = ops.tile([128, 512], f32, tag="ps", name="ps")
                    for c in range(8):
                        nc.tensor.matmul(
                            ps[:],
                            outcat[c][:, nj * 128:(nj + 1) * 128],
                            wo_t[c][:, ob * 512:(ob + 1) * 512],
                            start=(c == 0), stop=False)
                    nc.tensor.matmul(
                        ps[:],
                        ones_row[:, 0:128],
                        bo_sb[:, ob * 512:(ob + 1) * 512],
                        start=False, stop=True)
                    ost = osp.tile([128, 512], f16, tag="ost", name="ost")
                    nc.scalar.copy(ost[:], ps[:])
                    nc.sync.dma_start(
                        d_out[nj * 128:(nj + 1) * 128,
                              ob * 512:(ob + 1) * 512], ost[:])

            # attn_meanT: f16 accumulator DMA'd straight out
            for mj in range(8):
                nc.sync.dma_start(d_amT[mj * 128:(mj + 1) * 128, :],
                                  acc[mj][:])

        osp.release()
        wop.release()
        persist.release()
        const.release()

    nc.compile()
    _CACHE["nc"] = nc
    return nc


def prepare_in_maps(query, key, value, w_q, b_q, w_k, b_k, w_v, b_v, w_o, b_o):
    """Host-side sharding + layout prep. Returns list of per-core input dicts."""
    f = np.float32
    h16 = np.float16
    wqT = np.ascontiguousarray(np.asarray(w_q, f).T.astype(h16))
    wkT = np.ascontiguousarray(np.asarray(w_k, f).T.astype(h16))
    wvT = np.ascontiguousarray(np.asarray(w_v, f).T.astype(h16))
    woT16 = np.ascontiguousarray(
        (np.asarray(w_o, f).T * np.float32(H)).astype(h16))
    bq_pp = np.ascontiguousarray(np.asarray(b_q, f).reshape(8, 128).T)
    bk_pp = np.ascontiguousarray(np.asarray(b_k, f).reshape(8, 128).T)
    bv_r = np.asarray(b_v, f).reshape(1, D).astype(h16)
    bo_r = np.asarray(b_o, f).reshape(1, D).astype(h16)
    ones_r = np.ones((1, 512), h16)
    query = np.asarray(query, f)
    key = np.asarray(key, f)
    value = np.asarray(value, f)

    in_maps = []
    for b in range(B):
        in_maps.append({
            "qT_in": np.ascontiguousarray(query[b].T.astype(h16)),
            "kT_in": np.ascontiguousarray(key[b].T.astype(h16)),
            "vT_in": np.ascontiguousarray(value[b].T.astype(h16)),
            "wqT": wqT, "wkT": wkT, "wvT": wvT, "woT16": woT16,
            "bq_pp": bq_pp, "bk_pp": bk_pp, "bv_r": bv_r, "bo_r": bo_r,
            "ones_r": ones_r,
        })
    return in_maps


def run(in_maps, trace=False, **kw):
    _install_ntff_hook()
    from concourse.bass_utils import run_bass_kernel_spmd
    nc = build_module()
    return run_bass_kernel_spmd(nc, in_maps, core_ids=list(range(N_CORES)),
                                trace=trace, **kw)


def kernel(query, key, value, w_q, b_q, w_k, b_k, w_v, b_v, w_o, b_o):
    in_maps = prepare_in_maps(query, key, value, w_q, b_q, w_k, b_k,
                              w_v, b_v, w_o, b_o)
    res = run(in_maps)
    out = np.stack(
        [res.results[b]["out"].astype(np.float32) for b in range(B)])
    attn_mean = np.stack(
        [res.results[b]["attn_meanT"].T.astype(np.float32) for b in range(B)])
    return out, attn_mean


# revision 31
# speedup vs baseline: 1.3474x; 1.0122x over previous
"""Trainium2 Bass kernel for nn_CrossAttention (B=8, N=M=1024, D=1024, DK=768, H=16).

Sharding: data-parallel over batch B=8 -> one batch item per NeuronCore.
No collectives; attn.mean(dim=1) is over heads, all heads of a batch item
live on one core.

Per-core layout strategy:
  - Host pre-transposes activations and weights so every matmul has its
    contraction dim on SBUF partitions.
  - Scores are computed transposed: scoresT[m, n]. exp() is applied without
    max-subtraction (scores are bounded ~|2|; softmax value is unchanged).
  - QK^T for a head pair runs row-packed (64-row groups at tile_position
    (0,0)/(64,0)) so the two matmuls share the PE array concurrently.
  - PV uses a ones-augmented stationary [v_h | 1] so psum row 64 yields the
    softmax denominators for free.
  - The head-pair loop is software-pipelined: window p issues QK(p)
    interleaved with PV(p-1) at m-tile granularity to keep the PE stream
    dense; normalization/mean work of pair p-1 drains during window p.
  - exp tiles are combined [128, 2048] (head A cols 0:1024 | head B
    1024:2048) so the attn-mean mul/add chain runs half as many, twice as
    wide DVE ops; mj=7's chain runs on gpsimd to offload the DVE.
  - Softmax denominators: psum Z rows -> [1,2048] sbuf (ACT), DMA-reshaped
    to [128,16] so the reciprocal uses all 128 lanes, DMA'd back to a row,
    then one gpsimd partition_broadcast of the combined [128,2048] rbc.
  - outcat normalization is fused into the PSUM->SBUF eviction via DVE
    scalar_tensor_tensor (pv * rbc -> f16), replacing ACT copy + DVE mul.
  - attn_meanT accumulates per-head-half in acc2 [128,2048] f16; the final
    A+B merge writes f16 directly to DRAM; the host transposes and casts.
  - out = (outcatT * rbc16) @ (16*w_o.T) + b_o  (1/16 folded into the
    normalizer is compensated by scaling w_o.T by 16 on the host).
"""

import sys
import types

sys.path.insert(0, "/opt/trn_rl_repo")
sys.path.insert(0, "/root/.axon_site")

import numpy as np

N_CORES = 8
B, N, M = 8, 1024, 1024
D = 1024      # Q_DIM
DK = 768      # K_DIM
H = 16        # heads
HD = 64       # head dim
SCALE = HD ** -0.5  # 0.125


def _install_ntff_hook():
    """Make trace=True work under axon (antenv.axon_hooks shim)."""
    if "antenv.axon_hooks" in sys.modules:
        return
    try:
        import antenv
        hooks_mod = types.ModuleType("antenv.axon_hooks")
        _hook = [None]
        hooks_mod.set_axon_ntff_profile_hook = lambda h: _hook.__setitem__(0, h)
        hooks_mod.get_axon_ntff_profile_hook = lambda: _hook[0]
        sys.modules["antenv.axon_hooks"] = hooks_mod
        antenv.axon_hooks = hooks_mod
        from trn_agent_boot.trn_boot import _ntff_profile_via_ctypes
        hooks_mod.set_axon_ntff_profile_hook(
            _ntff_profile_via_ctypes("/opt/axon/libaxon_pjrt.so")
        )
    except Exception:
        pass


_CACHE = {}


def build_module():
    if "nc" in _CACHE:
        return _CACHE["nc"]

    import concourse.tile as tile
    import concourse.mybir as mybir
    from concourse import bacc, library_config

    f32 = mybir.dt.float32
    f32r = mybir.dt.float32r
    f16 = mybir.dt.float16
    AF = mybir.ActivationFunctionType
    ALU = mybir.AluOpType

    nc = bacc.Bacc("TRN2", target_bir_lowering=False, debug=False,
                   num_devices=N_CORES)

    # ---- DRAM tensors (per-core shard) ----
    d_qT = nc.dram_tensor("qT_in", [D, N], f16, kind="ExternalInput").ap()
    d_kT = nc.dram_tensor("kT_in", [DK, M], f16, kind="ExternalInput").ap()
    d_vT = nc.dram_tensor("vT_in", [DK, M], f16, kind="ExternalInput").ap()
    d_wqT = nc.dram_tensor("wqT", [D, D], f16, kind="ExternalInput").ap()
    d_wkT = nc.dram_tensor("wkT", [DK, D], f16, kind="ExternalInput").ap()
    d_wvT = nc.dram_tensor("wvT", [DK, D], f16, kind="ExternalInput").ap()
    d_woT = nc.dram_tensor("woT16", [D, D], f16, kind="ExternalInput").ap()
    d_bq = nc.dram_tensor("bq_pp", [128, 8], f32, kind="ExternalInput").ap()
    d_bk = nc.dram_tensor("bk_pp", [128, 8], f32, kind="ExternalInput").ap()
    d_bv = nc.dram_tensor("bv_r", [1, D], f16, kind="ExternalInput").ap()
    d_bo = nc.dram_tensor("bo_r", [1, D], f16, kind="ExternalInput").ap()
    d_ones = nc.dram_tensor("ones_r", [1, 512], f16, kind="ExternalInput").ap()
    d_out = nc.dram_tensor("out", [N, D], f16, kind="ExternalOutput").ap()
    d_amT = nc.dram_tensor("attn_meanT", [M, N], f16, kind="ExternalOutput").ap()

    with tile.TileContext(nc) as tc:
        nc.gpsimd.load_library(library_config.attn)

        # ---------------- persistent pools ----------------
        const = tc.alloc_tile_pool(name="const", bufs=1)
        persist = tc.alloc_tile_pool(name="persist", bufs=1)

        ones_row = const.tile([1, 128], f16, tag="ones", name="ones")
        nc.sync.dma_start(ones_row[:], d_ones[:, 0:128])
        bq_sb = const.tile([128, 8], f32, tag="bq", name="bq")
        bk_sb = const.tile([128, 8], f32, tag="bk", name="bk")

        qT = [persist.tile([128, N], f16, tag=f"qT{j}", name=f"qT{j}")
              for j in range(8)]
        kT = [persist.tile([128, M], f16, tag=f"kT{j}", name=f"kT{j}")
              for j in range(8)]
        v_sb = [persist.tile([128, H, HD + 1], f16, tag=f"v{j}", name=f"v{j}")
                for j in range(8)]
        outcat = [persist.tile([128, N], f16, tag=f"oc{j}", name=f"oc{j}")
                  for j in range(8)]
        acc = [persist.tile([128, N], f16, tag=f"acc{j}", name=f"acc{j}")
               for j in range(8)]

        # ones column scaled by H so the PV row-64 sums come out as H*Z and
        # their reciprocal is directly r = 1/(H*Z) (w_o is pre-scaled by H).
        for j in range(8):
            nc.vector.memset(v_sb[j][:, :, HD:HD + 1], float(H))

        # ---------------- phase 1: projections ----------------
        wkv = tc.alloc_tile_pool(name="wkv", bufs=1)
        wk_t = [wkv.tile([128, D], f16, tag=f"wk{c}", name=f"wk{c}")
                for c in range(6)]
        wv_t = [wkv.tile([128, D], f16, tag=f"wv{c}", name=f"wv{c}")
                for c in range(6)]
        bv_sb = wkv.tile([1, D], f16, tag="bv", name="bv")

        wqp = tc.alloc_tile_pool(name="wqp", bufs=1)
        wq_t = [wqp.tile([128, D], f16, tag=f"wq{c}", name=f"wq{c}")
                for c in range(8)]

        # --- Q projection: qT[do, n] = wqT-chunks.T @ qT_in ---
        with tc.tile_pool(name="xq", bufs=4) as xp, \
             tc.tile_pool(name="proj_ps", bufs=1, space="PSUM") as pps:
            for nb in range(2):
                pss = [pps.tile([128, 512], f32, tag=f"ps{j}", name=f"ps{j}")
                       for j in range(8)]
                for c in range(8):
                    if nb == 0:
                        nc.sync.dma_start(wq_t[c][:],
                                          d_wqT[c * 128:(c + 1) * 128, :])
                    xt = xp.tile([128, 512], f16, tag="x", name="xt")
                    nc.sync.dma_start(
                        xt[:], d_qT[c * 128:(c + 1) * 128,
                                    nb * 512:(nb + 1) * 512])
                    for j in range(8):
                        nc.tensor.matmul(
                            pss[j][:],
                            wq_t[c][:, j * 128:(j + 1) * 128],
                            xt[:],
                            start=(c == 0), stop=(c == 7))
                if nb == 0:
                    nc.sync.dma_start(bq_sb[:], d_bq[:, :])
                    nc.sync.dma_start(bk_sb[:], d_bk[:, :])
                for j in range(8):
                    nc.scalar.activation(
                        qT[j][:, nb * 512:(nb + 1) * 512], pss[j][:],
                        AF.Identity, bias=bq_sb[:, j:j + 1], scale=1.0)
        wqp.release()


        with tc.tile_pool(name="xkv", bufs=4) as xp, \
             tc.tile_pool(name="proj_ps2", bufs=1, space="PSUM") as pps:
            # --- K projection ---
            for nb in range(2):
                pss = [pps.tile([128, 512], f32, tag=f"ps{j}", name=f"ps{j}")
                       for j in range(8)]
                for c in range(6):
                    if nb == 0:
                        nc.sync.dma_start(wk_t[c][:],
                                          d_wkT[c * 128:(c + 1) * 128, :])
                        nc.sync.dma_start(wv_t[c][:],
                                          d_wvT[c * 128:(c + 1) * 128, :])
                    xt = xp.tile([128, 1024], f16, tag="x", name="xt")
                    nc.sync.dma_start(
                        xt[:, 0:512], d_kT[c * 128:(c + 1) * 128,
                                           nb * 512:(nb + 1) * 512])
                    for j in range(8):
                        nc.tensor.matmul(
                            pss[j][:],
                            wk_t[c][:, j * 128:(j + 1) * 128],
                            xt[:, 0:512],
                            start=(c == 0), stop=(c == 5))
                if nb == 0:
                    nc.sync.dma_start(bv_sb[:], d_bv[:, :])
                for j in range(8):
                    nc.scalar.activation(
                        kT[j][:, nb * 512:(nb + 1) * 512], pss[j][:],
                        AF.Identity, bias=bk_sb[:, j:j + 1], scale=1.0)

            # --- V projection: v[m, do] = vT_in-chunks.T @ wvT (+ b_v) ---
            for ob in range(2):
                pss = [pps.tile([128, 512], f32, tag=f"ps{j}", name=f"ps{j}")
                       for j in range(8)]
                for c in range(6):
                    xt = xp.tile([128, 1024], f16, tag="x", name="xt")
                    nc.sync.dma_start(xt[:], d_vT[c * 128:(c + 1) * 128, :])
                    for mj in range(8):
                        nc.tensor.matmul(
                            pss[mj][:],
                            xt[:, mj * 128:(mj + 1) * 128],
                            wv_t[c][:, ob * 512:(ob + 1) * 512],
                            start=(c == 0), stop=False)
                for mj in range(8):
                    nc.tensor.matmul(
                        pss[mj][:],
                        ones_row[:, 0:128],
                        bv_sb[:, ob * 512:(ob + 1) * 512],
                        start=False, stop=True)
                    nc.scalar.activation(
                        v_sb[mj][:, ob * 8:(ob + 1) * 8, 0:HD],
                        pss[mj][:].rearrange("p (a b) -> p a b", a=8),
                        AF.Copy)
        wkv.release()

        # w_o loaded during attention so O-projection starts without a stall
        wop = tc.alloc_tile_pool(name="wo", bufs=1)
        wo_t = [wop.tile([128, D], f16, tag=f"wo{c}", name=f"wo{c}")
                for c in range(8)]
        bo_sb = wop.tile([1, D], f16, tag="bo", name="bo")
        nc.sync.dma_start(bo_sb[:], d_bo[:, :])
        for c in range(8):
            nc.sync.dma_start(wo_t[c][:], d_woT[c * 128:(c + 1) * 128, :])

        # O-proj staging pool allocated BEFORE attention pools so its SBUF
        # does not overlap exp tiles (would serialize O-proj behind the last
        # pair's mean ops).
        osp = tc.alloc_tile_pool(name="ostage", bufs=2)

        # ---------------- phase 2: attention (software-pipelined pairs) ----
        # Pipeline (window p): QK(p)+exp(p); PV(p-1) front-loaded so its Z
        # row is ready mid-window; recip(p-1) mid-window on DVE; bc(p-1) on
        # gpsimd late-window; means(p-2) run from window top (their rbc was
        # finished a window ago).  exp tags for the late-draining mj get a
        # third buffer so next-next window's exp doesn't stall on the mean
        # backlog.
        with tc.tile_pool(name="exp", bufs=2) as expp, \
             tc.tile_pool(name="att_tmp", bufs=1) as tmpp, \
             tc.tile_pool(name="rinvp", bufs=1) as rinvp, \
             tc.tile_pool(name="rbc", bufs=2) as rbcp, \
             tc.tile_pool(name="qk_ps", bufs=2, space="PSUM") as qkps, \
             tc.tile_pool(name="pv_ps", bufs=1, space="PSUM") as pvps:

            exps = {}
            pvts = {}
            rbcs = {}

            def emit_qk_mj(p, mj):
                tA = qkps.tile([128, 1024], f32, tag="qk", name="qkA")
                tB = qkps.tile([128, 1024], f32, tag="qk", name="qkB")
                for nb in range(2):
                    nc.tensor.matmul(
                        tA[:, nb * 512:(nb + 1) * 512],
                        kT[p][0:64, mj * 128:(mj + 1) * 128],
                        qT[p][0:64, nb * 512:(nb + 1) * 512],
                        start=True, stop=True, tile_position=(0, 0))
                    nc.tensor.matmul(
                        tB[:, nb * 512:(nb + 1) * 512],
                        kT[p][64:128, mj * 128:(mj + 1) * 128],
                        qT[p][64:128, nb * 512:(nb + 1) * 512],
                        start=True, stop=True, tile_position=(64, 0))
                et = expp.tile([128, 2 * N], f16, tag=f"exp{mj}",
                               name=f"exp{mj}",
                               bufs=3 if mj in (5, 6, 7) else 2)
                nc.scalar.activation(et[:, 0:N], tA[:], AF.Exp, scale=SCALE)
                nc.scalar.activation(et[:, N:2 * N], tB[:], AF.Exp,
                                     scale=SCALE)
                exps[p].append(et)

            def emit_pv(q):
                pv = pvps.tile([65, 2048], f32, tag="pv", name="pv")
                pvts[q] = pv
                cq = exps[q]
                for mj in range(8):
                    for nb in range(2):
                        nc.tensor.matmul(
                            pv[:, nb * 512:(nb + 1) * 512],
                            v_sb[mj][:, 2 * q, :],
                            cq[mj][:, nb * 512:(nb + 1) * 512],
                            start=(mj == 0), stop=(mj == 7))
                        nc.tensor.matmul(
                            pv[:, N + nb * 512:N + (nb + 1) * 512],
                            v_sb[mj][:, 2 * q + 1, :],
                            cq[mj][:, N + nb * 512:N + (nb + 1) * 512],
                            start=(mj == 0), stop=(mj == 7))

            def emit_mean(q, mj):
                at = tmpp.tile([128, 2 * N], f16, tag="at", name="at")
                nc.vector.tensor_mul(at[:], exps[q][mj][:], rbcs[q][:])
                if q == 0:
                    nc.vector.tensor_add(acc[mj][:], at[:, 0:N],
                                         at[:, N:2 * N])
                else:
                    nc.vector.tensor_add(acc[mj][:], acc[mj][:], at[:, 0:N])
                    nc.vector.tensor_add(acc[mj][:], acc[mj][:],
                                         at[:, N:2 * N])

            def emit_norms(q):
                # in-place normalization of the evicted outcat (cheap f16 2x)
                nc.vector.tensor_mul(outcat[q][0:64, :], outcat[q][0:64, :],
                                     rbcs[q][0:64, 0:N])
                nc.vector.tensor_mul(outcat[q][64:128, :],
                                     outcat[q][64:128, :],
                                     rbcs[q][64:128, N:2 * N])

            for p in range(10):
                if p < 8:
                    exps[p] = []
                    emit_qk_mj(p, 0)
                    emit_qk_mj(p, 1)
                if 1 <= p <= 8:
                    emit_pv(p - 1)
                if p < 8:
                    for mj in range(2, 6):
                        emit_qk_mj(p, mj)

                # drain of pair p-1: Z row -> all-lane reciprocal -> rbc.
                # The ACT row-copy is emitted mid-exp-stream so the DVE
                # reciprocal's input is ready when the DVE FIFO reaches it.
                if 1 <= p <= 8:
                    q = p - 1
                    pv = pvts[q]
                    sAB = rinvp.tile([1, 2048], f32, tag="sAB", name="sAB")
                    z128 = rinvp.tile([128, 16], f32, tag="z128", name="z128")
                    ri128 = rinvp.tile([128, 16], f32, tag="ri128",
                                       name="ri128")
                    r16c = rinvp.tile([128, 16], f16, tag="r16c", name="r16c")
                    r16row = rinvp.tile([1, 2048], f16, tag="r16row",
                                        name="r16row")
                    nc.scalar.copy(sAB[:], pv[64:65, :])
                    nc.sync.dma_start(z128[:], sAB[:])
                    # unnormalized outcat eviction (normalized next window)
                    nc.scalar.copy(outcat[q][0:64, :], pv[0:64, 0:N])
                    nc.scalar.copy(outcat[q][64:128, :], pv[0:64, N:2 * N])
                    del pvts[q]

                if p < 8:
                    for mj in range(6, 8):
                        emit_qk_mj(p, mj)

                # means of pair p-2 (their rbc completed last window);
                # earliest-mj first — they gate this window's exp slots
                if p >= 2:
                    emit_mean(p - 2, 0)
                    emit_mean(p - 2, 1)
                    emit_norms(p - 2)
                    for mj in range(2, 6):
                        emit_mean(p - 2, mj)

                if 1 <= p <= 8:
                    q = p - 1
                    nc.vector.reciprocal_approx_fast(out=ri128[:],
                                                     in_=z128[:])
                    nc.vector.tensor_scalar_mul(r16c[:], ri128[:], 1.0)
                    nc.sync.dma_start(r16row[:], r16c[:])
                    rbc2 = rbcp.tile([128, 2 * N], f16, tag="rbc2",
                                     name="rbc2")
                    nc.gpsimd.partition_broadcast(rbc2[:], r16row[:])
                    rbcs[q] = rbc2

                if p >= 2:
                    for mj in range(6, 8):
                        emit_mean(p - 2, mj)
                    del exps[p - 2]

        # ---------------- phase 3: O-projection + outputs ----------------
        with tc.tile_pool(name="o_ps", bufs=4, space="PSUM") as ops:
            for nj in range(8):
                for ob in range(2):
                    ps = ops.tile([128, 512], f32, tag="ps", name="ps")
                    for c in range(8):
                        nc.tensor.matmul(
                            ps[:],
                            outcat[c][:, nj * 128:(nj + 1) * 128],
                            wo_t[c][:, ob * 512:(ob + 1) * 512],
                            start=(c == 0), stop=False)
                    nc.tensor.matmul(
                        ps[:],
                        ones_row[:, 0:128],
                        bo_sb[:, ob * 512:(ob + 1) * 512],
                        start=False, stop=True)
                    ost = osp.tile([128, 512], f16, tag="ost", name="ost")
                    nc.scalar.copy(ost[:], ps[:])
                    nc.sync.dma_start(
                        d_out[nj * 128:(nj + 1) * 128,
                              ob * 512:(ob + 1) * 512], ost[:])

            # attn_meanT: f16 accumulator DMA'd straight out
            for mj in range(8):
                nc.sync.dma_start(d_amT[mj * 128:(mj + 1) * 128, :],
                                  acc[mj][:])

        osp.release()
        wop.release()
        persist.release()
        const.release()

    nc.compile()
    _CACHE["nc"] = nc
    return nc


def prepare_in_maps(query, key, value, w_q, b_q, w_k, b_k, w_v, b_v, w_o, b_o):
    """Host-side sharding + layout prep. Returns list of per-core input dicts."""
    f = np.float32
    h16 = np.float16
    wqT = np.ascontiguousarray(np.asarray(w_q, f).T.astype(h16))
    wkT = np.ascontiguousarray(np.asarray(w_k, f).T.astype(h16))
    wvT = np.ascontiguousarray(np.asarray(w_v, f).T.astype(h16))
    woT16 = np.ascontiguousarray(
        (np.asarray(w_o, f).T * np.float32(H)).astype(h16))
    bq_pp = np.ascontiguousarray(np.asarray(b_q, f).reshape(8, 128).T)
    bk_pp = np.ascontiguousarray(np.asarray(b_k, f).reshape(8, 128).T)
    bv_r = np.asarray(b_v, f).reshape(1, D).astype(h16)
    bo_r = np.asarray(b_o, f).reshape(1, D).astype(h16)
    ones_r = np.ones((1, 512), h16)
    query = np.asarray(query, f)
    key = np.asarray(key, f)
    value = np.asarray(value, f)

    in_maps = []
    for b in range(B):
        in_maps.append({
            "qT_in": np.ascontiguousarray(query[b].T.astype(h16)),
            "kT_in": np.ascontiguousarray(key[b].T.astype(h16)),
            "vT_in": np.ascontiguousarray(value[b].T.astype(h16)),
            "wqT": wqT, "wkT": wkT, "wvT": wvT, "woT16": woT16,
            "bq_pp": bq_pp, "bk_pp": bk_pp, "bv_r": bv_r, "bo_r": bo_r,
            "ones_r": ones_r,
        })
    return in_maps


def run(in_maps, trace=False, **kw):
    _install_ntff_hook()
    from concourse.bass_utils import run_bass_kernel_spmd
    nc = build_module()
    return run_bass_kernel_spmd(nc, in_maps, core_ids=list(range(N_CORES)),
                                trace=trace, **kw)


def kernel(query, key, value, w_q, b_q, w_k, b_k, w_v, b_v, w_o, b_o):
    in_maps = prepare_in_maps(query, key, value, w_q, b_q, w_k, b_k,
                              w_v, b_v, w_o, b_o)
    res = run(in_maps)
    out = np.stack(
        [res.results[b]["out"].astype(np.float32) for b in range(B)])
    attn_mean = np.stack(
        [res.results[b]["attn_meanT"].T.astype(np.float32) for b in range(B)])
    return out, attn_mean
